# revision 1
# baseline (speedup 1.0000x reference)
"""Trainium2 Bass kernel for a 3-layer dense transformer (BigramModel).

Contract: kernel(**inputs) takes the FULL unsharded numpy inputs (as produced
by setup_inputs) and returns the full [B, T, V] float32 logits. Internally the
batch dim B=128 is sharded 16-per-core across 8 NeuronCores (pure data
parallelism, weights replicated), one Bass/Tile NEFF run via
run_bass_kernel_spmd.

Layout strategy on device (per core, 16 seqs x 256 tok = 4096 tokens):
  - residual h: token-major fp32 SBUF tiles [128, 384] x 32 (persistent)
  - LayerNorm: DVE bn_stats/bn_aggr per tile; rstd = exp(-0.5*ln(var+eps))
    (keeps ACT in the natural_log_exp table set shared with softmax exp);
    gamma/beta are folded into the adjacent weight matrices on the host.
  - matmuls in bf16 (fp32 PSUM accumulation). Feature-major operands
    (xn^T, o^T) produced by bf16 DMA transposes (XBAR).
  - attention: per (seq, head) scores kept feature-major [s, t] so softmax
    sums run through the matmul path: V is augmented with a ones column so
    the o-matmul also produces the softmax denominators; probs are masked
    multiplicatively after exp (no -inf handling needed).
  - biases that land on free dims (bproj, b2, beta@Wv) are added inside PSUM
    via K=1 ones-row matmuls, emitted only when the host sees nonzero values.
"""

import numpy as np
import ml_dtypes

BF16 = ml_dtypes.bfloat16

P = 128
T = 256
E = 384
V = 65
H = 6
HS = 64
FF = 1536
L = 3
NCORES = 8
BPC = 16              # sequences per core
TOK = BPC * T         # 4096 tokens per core
NT = TOK // P         # 32 token tiles
NB = TOK // 512       # 8 blocks of 512 tokens (2 seqs)
ECH = E // P          # 3
FCH = FF // P         # 12

_NC_CACHE = {}
TMODE = "dma"   # "dma" = XBAR dma transpose; "pe" = TensorE transpose + copy
STAGE = 99      # debug: truncate per-block body (1=LN,2=QKV,3=probs,4=o,5=proj,6=MLP)
SUB = 2         # debug stage-3 detail (legacy)
MLPVAR = "full" # debug: "reuse" skip 2nd LN, "norelu" plain evac, "full"


def _build_nc(flags):
    """Build + compile the Bass program. flags = (bv_nz, bp_nz, b2_nz) per layer."""
    import concourse.bacc as bacc
    import concourse.mybir as mybir
    import concourse.tile as tile

    dt = mybir.dt
    f32 = dt.float32
    bf = dt.bfloat16
    Alu = mybir.AluOpType
    Act = mybir.ActivationFunctionType

    from concourse.masks import make_identity

    nc = bacc.Bacc("TRN2", target_bir_lowering=False, debug=False, num_devices=1)

    # ---- DRAM tensors (shapes match SBUF layouts; host pre-arranges) ----
    D = {}
    D["oh"] = nc.dram_tensor("oh", [V, TOK], bf, kind="ExternalInput")
    D["te"] = nc.dram_tensor("te", [V, E], bf, kind="ExternalInput")
    D["pos"] = nc.dram_tensor("pos", [P, 2, E], f32, kind="ExternalInput")
    D["mask"] = nc.dram_tensor("mask", [P, 2 * P], bf, kind="ExternalInput")
    for l in range(L):
        for w in ("wq", "wk", "wv", "wproj"):
            D[f"{w}{l}"] = nc.dram_tensor(f"{w}{l}", [P, ECH, E], bf, kind="ExternalInput")
        D[f"bq{l}"] = nc.dram_tensor(f"bq{l}", [P, ECH], f32, kind="ExternalInput")
        D[f"bk{l}"] = nc.dram_tensor(f"bk{l}", [P, ECH], f32, kind="ExternalInput")
        D[f"w1{l}"] = nc.dram_tensor(f"w1{l}", [P, ECH, FF], bf, kind="ExternalInput")
        D[f"b1c{l}"] = nc.dram_tensor(f"b1c{l}", [P, FCH], f32, kind="ExternalInput")
        D[f"w2{l}"] = nc.dram_tensor(f"w2{l}", [P, FCH, E], bf, kind="ExternalInput")
        D[f"bvrow{l}"] = nc.dram_tensor(f"bvrow{l}", [1, E], bf, kind="ExternalInput")
        D[f"bpc{l}"] = nc.dram_tensor(f"bpc{l}", [P, ECH], f32, kind="ExternalInput")
        D[f"b2c{l}"] = nc.dram_tensor(f"b2c{l}", [P, ECH], f32, kind="ExternalInput")
    if MLPVAR == "w2dump":
        D["dbga"] = nc.dram_tensor("dbga", [P, FCH, 512], bf, kind="ExternalOutput")
        D["dbgo"] = nc.dram_tensor("dbgo", [P, 4, E], f32, kind="ExternalOutput")
    D["wout"] = nc.dram_tensor("wout", [P, ECH, V], bf, kind="ExternalInput")
    D["boutc"] = nc.dram_tensor("boutc", [V, 1], f32, kind="ExternalInput")
    D["logT"] = nc.dram_tensor("logT", [V, TOK], f32, kind="ExternalOutput")

    bv_nz, bp_nz, b2_nz = flags

    with tile.TileContext(nc) as tc:
        import contextlib

        with contextlib.ExitStack() as ctx:
            const = ctx.enter_context(tc.tile_pool(name="const", bufs=1))
            wpool = ctx.enter_context(tc.tile_pool(name="wpool", bufs=1))
            act = ctx.enter_context(tc.tile_pool(name="act", bufs=3))
            act2 = ctx.enter_context(tc.tile_pool(name="act2", bufs=2))
            act1 = ctx.enter_context(tc.tile_pool(name="act1", bufs=1))
            ps_lin = ctx.enter_context(tc.tile_pool(name="ps_lin", bufs=4, space="PSUM"))
            ps_sc = ctx.enter_context(tc.tile_pool(name="ps_sc", bufs=3, space="PSUM"))

            def load_const(name, shape, dtp):
                t = const.tile(shape, dtp, tag=name)
                nc.sync.dma_start(out=t[:], in_=D[name].ap())
                return t

            # pad the K=65 embedding contraction to K=128 (sub-128 partition
            # matmuls are flaky on HW); pad rows are zeroed so they add 0.
            oh_sb = const.tile([P, TOK], bf, tag="oh")
            nc.vector.memset(oh_sb[:], 0.0)
            nc.sync.dma_start(out=oh_sb[0:V, :], in_=D["oh"].ap())
            te_sb = const.tile([P, E], bf, tag="te")
            nc.vector.memset(te_sb[:], 0.0)
            nc.sync.dma_start(out=te_sb[0:V, :], in_=D["te"].ap())
            pos_sb = load_const("pos", [P, 2, E], f32)
            mask_sb = load_const("mask", [P, 2 * P], bf)
            boutc_sb = load_const("boutc", [V, 1], f32)
            ones_sb = const.tile([1, P], bf, tag="ones")
            nc.vector.memset(ones_sb[:], 1.0)
            eps_sb = const.tile([P, 1], f32, tag="eps")
            nc.vector.memset(eps_sb[:], 1e-5)
            zero_sb = const.tile([P, 1], f32, tag="zero")
            nc.vector.memset(zero_sb[:], 0.0)
            if TMODE == "pe":
                ident_sb = const.tile([P, P], bf, tag="ident")
                make_identity(nc, ident_sb[:])
            _tp_ctr = [0]

            def tpose(dst, src):
                """dst[P,128] (sbuf bf16) = transpose(src[P,128] sbuf bf16)."""
                if TMODE == "dma":
                    nc.sync.dma_start_transpose(dst, src)
                    return
                tp = ps_lin.tile([P, P], f32, tag="mm", name="tp")
                nc.tensor.transpose(tp[:], src, ident_sb[:])
                k = _tp_ctr[0] = _tp_ctr[0] + 1
                if k % 2 == 0:
                    nc.vector.tensor_copy(out=dst, in_=tp[:])
                else:
                    nc.scalar.copy(out=dst, in_=tp[:])

            # persistent residual tiles
            h = []
            for i in range(NT):
                h.append(const.tile([P, E], f32, tag=f"h{i}", name=f"h{i}"))

            # ---- embedding: h = onehot.T @ tok_emb + pos ----
            for i in range(NT):
                ps = ps_lin.tile([P, E], f32, tag="mm")
                nc.tensor.matmul(
                    ps[:], oh_sb[:, i * P:(i + 1) * P], te_sb[:],
                    start=True, stop=True,
                )
                nc.vector.tensor_add(out=h[i][:], in0=ps[:], in1=pos_sb[:, i % 2, :])

            def ln_block(i0, tag):
                """LN of h[i0..i0+3] -> xn bf16 [P,4,E] and xnT bf16 [P,ECH,512]."""
                xn = act2.tile([P, 4, E], bf, tag="xn")
                mv4 = act.tile([P, 4, 2], f32, tag="mv")
                rstd4 = act.tile([P, 4], f32, tag="rstd")
                for j in range(4):
                    st6 = act.tile([P, 6], f32, tag="bnst")
                    nc.vector.bn_stats(out=st6[:], in_=h[i0 + j][:])
                    nc.vector.bn_aggr(out=mv4[:, j, :], in_=st6[:])
                # rstd = exp(-0.5 * ln(var + eps))
                nc.scalar.activation(
                    out=rstd4[:], in_=mv4[:, :, 1], func=Act.Ln, bias=eps_sb[:],
                )
                nc.scalar.activation(
                    out=rstd4[:], in_=rstd4[:], func=Act.Exp, scale=-0.5,
                )
                for j in range(4):
                    nc.vector.tensor_scalar(
                        out=xn[:, j, :], in0=h[i0 + j][:],
                        scalar1=mv4[:, j, 0:1], scalar2=rstd4[:, j:j + 1],
                        op0=Alu.subtract, op1=Alu.mult,
                    )
                xnT = act.tile([P, ECH, 512], bf, tag="xnT")
                for j in range(4):
                    for c in range(ECH):
                        tpose(
                            xnT[:, c, j * P:(j + 1) * P],
                            xn[:, j, c * P:(c + 1) * P],
                        )
                return xnT

            def linear_fmaj(xnT, w_sb, bias_sb, fch, tag, relu=False):
                """feature-major out [P, fch, 512] bf16 = (W^T xn^T); bias per-partition."""
                o = (act1 if fch == FCH else act2).tile([P, fch, 512], bf, tag=tag, name=tag)
                for f in range(fch):
                    ps = ps_lin.tile([P, 512], f32, tag="mm")
                    for c in range(ECH):
                        nc.tensor.matmul(
                            ps[:], w_sb[:, c, f * P:(f + 1) * P], xnT[:, c, :],
                            start=(c == 0), stop=(c == ECH - 1),
                        )
                    if relu:
                        nc.vector.tensor_scalar(
                            out=o[:, f, :], in0=ps[:],
                            scalar1=bias_sb[:, f:f + 1], scalar2=zero_sb[:],
                            op0=Alu.add, op1=Alu.max,
                        )
                    elif bias_sb is not None:
                        nc.vector.tensor_scalar_add(
                            out=o[:, f, :], in0=ps[:], scalar1=bias_sb[:, f:f + 1],
                        )
                    else:
                        nc.vector.tensor_copy(out=o[:, f, :], in_=ps[:])
                return o

            def linear_fmaj_resid(xT, w_sb, nch, bias_col, i0, tag):
                """h[i0+j] += (W^T x)_j via the feature-major matmul pattern
                (weights as lhsT), then DMA-transpose back to token-major."""
                yT = act2.tile([P, ECH, 512], bf, tag="yT", name="yT")
                for f in range(ECH):
                    ps = ps_lin.tile([P, 512], f32, tag="mm")
                    for c in range(nch):
                        nc.tensor.matmul(
                            ps[:], w_sb[:, c, f * P:(f + 1) * P], xT[:, c, :],
                            start=(c == 0), stop=(c == nch - 1),
                        )
                    nc.vector.tensor_scalar_add(
                        out=yT[:, f, :], in0=ps[:], scalar1=bias_col[:, f:f + 1])
                ytm = act2.tile([P, 4, E], bf, tag="ytm", name="ytm")
                for j in range(4):
                    for c in range(ECH):
                        tpose(
                            ytm[:, j, c * P:(c + 1) * P],
                            yT[:, c, j * P:(j + 1) * P],
                        )
                for j in range(4):
                    nc.vector.tensor_add(
                        out=h[i0 + j][:], in0=h[i0 + j][:], in1=ytm[:, j, :])

            def load_w(name, shape, dtp):
                t = wpool.tile(shape, dtp, tag=name[:-1])  # tag without layer idx
                nc.sync.dma_start(out=t[:], in_=D[name].ap())
                return t

            # ---- transformer layers ----
            for l in range(L):
                wq = load_w(f"wq{l}", [P, ECH, E], bf)
                wk = load_w(f"wk{l}", [P, ECH, E], bf)
                wv = load_w(f"wv{l}", [P, ECH, E], bf)
                wproj = load_w(f"wproj{l}", [P, ECH, E], bf)
                bq = load_w(f"bq{l}", [P, ECH], f32)
                bk = load_w(f"bk{l}", [P, ECH], f32)
                w1 = load_w(f"w1{l}", [P, ECH, FF], bf)
                b1c = load_w(f"b1c{l}", [P, FCH], f32)
                w2 = load_w(f"w2{l}", [P, FCH, E], bf)
                bvrow = load_w(f"bvrow{l}", [1, E], bf) if bv_nz[l] else None
                bpc = load_w(f"bpc{l}", [P, ECH], f32)
                b2c = load_w(f"b2c{l}", [P, ECH], f32)

                for b in range(NB):
                    i0 = 4 * b
                    # --- attention sublayer ---
                    if STAGE < 1:
                        continue
                    xnT = ln_block(i0, "a")
                    if STAGE < 2:
                        continue
                    QT = linear_fmaj(xnT, wq, bq, ECH, "QT")
                    KT = linear_fmaj(xnT, wk, bk, ECH, "KT")
                    # V token-major, ones-augmented: [P, 4, H, 65]
                    Vt = act2.tile([P, 4, H, 65], bf, tag="Vt")
                    for j in range(4):
                        ps = ps_lin.tile([P, E], f32, tag="mm")
                        for c in range(ECH):
                            nc.tensor.matmul(
                                ps[:], xnT[:, c, j * P:(j + 1) * P], wv[:, c, :],
                                start=(c == 0),
                                stop=(c == ECH - 1 and bvrow is None),
                            )
                        if bvrow is not None:
                            nc.tensor.matmul(
                                ps[:], ones_sb[:], bvrow[:], start=False, stop=True,
                            )
                        nc.vector.tensor_copy(
                            out=Vt[:, j, :, 0:64],
                            in_=ps.rearrange("p (h d) -> p h d", h=H),
                        )
                        nc.vector.memset(Vt[:, j, :, 64:65], 1.0)

                    if STAGE < 3:
                        continue
                    oT = act2.tile([P, ECH, 512], bf, tag="oT")
                    for s in range(2):      # the 2 sequences in this block
                        tb = s * 256        # col offset within the 512 block
                        probs = act2.tile([P, 2, H, 256], bf, tag="probs")
                        for st in range(2):  # s_tile (128 keys each)
                            tlo = 128 if st == 1 else 0
                            for hh in range(H):
                                c, off = divmod(hh * HS, P)
                                # each matmul gets its own offset-0 psum tile:
                                # outputs at nonzero tile offsets miscompute
                                # on HW (walrus bank mapping).
                                sc = ps_sc.tile([P, 256], f32, tag="sc", name="sc")
                                nc.tensor.matmul(
                                    sc[:, 0:256 - tlo],
                                    KT[off:off + HS, c, tb + st * P: tb + (st + 1) * P],
                                    QT[off:off + HS, c, tb + tlo: tb + 256],
                                    start=True, stop=True,
                                )
                                nc.scalar.activation(
                                    out=probs[:, st, hh, tlo:256],
                                    in_=sc[:, 0:256 - tlo],
                                    func=Act.Exp, scale=float(HS) ** -0.5,
                                )
                            if st == 0:
                                nc.vector.tensor_tensor(
                                    out=probs[:, 0], in0=probs[:, 0],
                                    in1=mask_sb[:, None, :].to_broadcast((P, H, 256)),
                                    op=Alu.mult,
                                )
                            else:
                                nc.vector.tensor_tensor(
                                    out=probs[:, 1, :, P:256],
                                    in0=probs[:, 1, :, P:256],
                                    in1=mask_sb[:, None, 0:P].to_broadcast((P, H, P)),
                                    op=Alu.mult,
                                )
                        if STAGE < 4:
                            continue
                        onorm = act2.tile([P, 2, E], bf, tag="onorm")
                        for tt in range(2):  # query tiles of this seq
                            # one single-shot matmul per (head, s-chunk), each
                            # into its own offset-0 psum tile; combine in SBUF.
                            osum = act2.tile([P, H, 65], f32, tag="osum")
                            for hh in range(H):
                                oa = ps_lin.tile([P, 65], f32, tag="mm", name="oa")
                                nc.tensor.matmul(
                                    oa[:],
                                    probs[:, 0, hh, tt * P:(tt + 1) * P],
                                    Vt[:, 2 * s, hh, :],
                                    start=True, stop=True,
                                )
                                nc.scalar.copy(out=osum[:, hh, :], in_=oa[:])
                                if tt == 1:
                                    oab = ps_lin.tile([P, 65], f32, tag="mm", name="oab")
                                    nc.tensor.matmul(
                                        oab[:],
                                        probs[:, 1, hh, P:2 * P],
                                        Vt[:, 2 * s + 1, hh, :],
                                        start=True, stop=True,
                                    )
                                    nc.vector.tensor_add(
                                        out=osum[:, hh, :], in0=osum[:, hh, :],
                                        in1=oab[:])
                            rec = act.tile([P, H], f32, tag="rec")
                            nc.vector.reciprocal(out=rec[:], in_=osum[:, :, 64])
                            nc.vector.tensor_tensor(
                                out=onorm[:, tt].rearrange("p (h d) -> p h d", h=H),
                                in0=osum[:, :, 0:64],
                                in1=rec[:, :, None].to_broadcast((P, H, HS)),
                                op=Alu.mult,
                            )
                        for tt in range(2):
                            for c in range(ECH):
                                tpose(
                                    oT[:, c, (2 * s + tt) * P:(2 * s + tt + 1) * P],
                                    onorm[:, tt, c * P:(c + 1) * P],
                                )
                    if STAGE < 5:
                        continue
                    linear_fmaj_resid(oT, wproj, ECH, bpc, i0, "p")

                    # --- MLP sublayer ---
                    if STAGE < 6:
                        continue
                    xnT2 = xnT if MLPVAR == "reuse" else ln_block(i0, "m")
                    aT = linear_fmaj(xnT2, w1, b1c, FCH, "aT",
                                     relu=(MLPVAR != "norelu"))
                    if MLPVAR == "w2dump":
                        nc.sync.dma_start(out=D["dbga"].ap(), in_=aT[:])
                        for j in range(4):
                            ps = ps_lin.tile([P, E], f32, tag="mm", name="psd")
                            for c in range(FCH):
                                lw = act.tile([P, P], bf, tag="lw", name="lw")
                                nc.vector.tensor_copy(
                                    out=lw[:], in_=aT[:, c, j * P:(j + 1) * P])
                                nc.tensor.matmul(
                                    ps[:], lw[:],
                                    w2[:, c, :],
                                    start=(c == 0), stop=(c == FCH - 1),
                                )
                            dtmp = act.tile([P, E], f32, tag="dtmp")
                            nc.vector.tensor_copy(out=dtmp[:], in_=ps[:])
                            nc.sync.dma_start(out=D["dbgo"].ap()[:, j, :], in_=dtmp[:])
                    elif MLPVAR == "dmastage":
                        aT2 = act1.tile([P, FCH, 512], bf, tag="aT2", name="aT2")
                        nc.sync.dma_start(out=aT2[:], in_=aT[:])
                        linear_fmaj_resid(aT2, w2, FCH, b2c, i0, "m")
                    elif MLPVAR != "w1only":
                        linear_fmaj_resid(aT, w2, FCH, b2c, i0, "m")

            # ---- final LN + unembed (feature-major logits) ----
            wout = wpool.tile([P, ECH, V], bf, tag="wout")
            nc.sync.dma_start(out=wout[:], in_=D["wout"].ap())
            for b in range(NB):
                xnfT = ln_block(4 * b, "f")
                ps = ps_lin.tile([V, 512], f32, tag="mm")
                for c in range(ECH):
                    nc.tensor.matmul(
                        ps[:], wout[:, c, :], xnfT[:, c, :],
                        start=(c == 0), stop=(c == ECH - 1),
                    )
                lt = act2.tile([V, 512], f32, tag="lt")
                nc.vector.tensor_scalar_add(out=lt[:], in0=ps[:], scalar1=boutc_sb[:])
                nc.sync.dma_start(
                    out=D["logT"].ap()[:, b * 512:(b + 1) * 512], in_=lt[:],
                )

    nc.compile()
    return nc


def _prep_shared(inp):
    """Host-side weight prep: layout rearrangement + LN gamma/beta folding."""
    sh = {}

    def f32(x):
        return np.asarray(x, np.float32)

    sh["te"] = np.asarray(f32(inp["tok_emb"]), BF16)                      # [V,E]
    sh["pos"] = np.ascontiguousarray(
        f32(inp["pos_emb"]).reshape(2, P, E).transpose(1, 0, 2))          # [P,2,E]
    m = np.concatenate(
        [np.triu(np.ones((P, P), np.float32)), np.ones((P, P), np.float32)], axis=1)
    sh["mask"] = np.asarray(m, BF16)                                      # [P,256]

    def tile3(w, fdim):  # [E, fdim] -> [P, ECH, fdim]
        return np.ascontiguousarray(w.reshape(ECH, P, fdim).transpose(1, 0, 2))

    def col(b, nch):  # [nch*P] -> [P, nch]
        return np.ascontiguousarray(b.reshape(nch, P).T)

    bv_nz, bp_nz, b2_nz = [], [], []
    for l in range(L):
        g1, b1_ = f32(inp["ln1_g"][l]), f32(inp["ln1_b"][l])
        g2, b2_ = f32(inp["ln2_g"][l]), f32(inp["ln2_b"][l])
        wq = f32(inp["Wq"][l]).transpose(1, 0, 2).reshape(E, E)   # head-major cols
        wk = f32(inp["Wk"][l]).transpose(1, 0, 2).reshape(E, E)
        wv = f32(inp["Wv"][l]).transpose(1, 0, 2).reshape(E, E)
        sh[f"wq{l}"] = np.asarray(tile3(g1[:, None] * wq, E), BF16)
        sh[f"wk{l}"] = np.asarray(tile3(g1[:, None] * wk, E), BF16)
        sh[f"wv{l}"] = np.asarray(tile3(g1[:, None] * wv, E), BF16)
        sh[f"bq{l}"] = col(wq.T @ b1_, ECH)
        sh[f"bk{l}"] = col(wk.T @ b1_, ECH)
        bv = wv.T @ b1_
        sh[f"bvrow{l}"] = np.asarray(bv[None, :], BF16)
        bv_nz.append(bool(np.any(bv != 0)))
        wp = f32(inp["Wproj"][l])
        sh[f"wproj{l}"] = np.asarray(tile3(wp, E), BF16)
        bp = f32(inp["bproj"][l])
        sh[f"bpc{l}"] = col(bp, ECH)
        bp_nz.append(bool(np.any(bp != 0)))
        w1 = f32(inp["W1"][l])
        sh[f"w1{l}"] = np.asarray(tile3(g2[:, None] * w1, FF), BF16)
        sh[f"b1c{l}"] = col(f32(inp["b1"][l]) + w1.T @ b2_, FCH)
        w2 = f32(inp["W2"][l])
        sh[f"w2{l}"] = np.asarray(
            w2.reshape(FCH, P, E).transpose(1, 0, 2), BF16)
        b2r = f32(inp["b2"][l])
        sh[f"b2c{l}"] = col(b2r, ECH)
        b2_nz.append(bool(np.any(b2r != 0)))

    gf, bf_ = f32(inp["lnf_g"]), f32(inp["lnf_b"])
    wo = f32(inp["Wout"])
    sh["wout"] = np.asarray(tile3(gf[:, None] * wo, V), BF16)
    sh["boutc"] = (f32(inp["bout"]) + wo.T @ bf_).reshape(V, 1)
    flags = (tuple(bv_nz), tuple(bp_nz), tuple(b2_nz))
    return sh, flags


def _onehot(xc):
    """xc: [BPC, T] ints -> [V, TOK] bf16 one-hot (feature-major)."""
    xf = np.asarray(xc, np.int64).reshape(-1)
    oh = np.zeros((V, TOK), np.float32)
    oh[xf, np.arange(TOK)] = 1.0
    return np.asarray(oh, BF16)


def _get_nc(flags):
    if flags not in _NC_CACHE:
        _NC_CACHE[flags] = _build_nc(flags)
    return _NC_CACHE[flags]


def make_in_maps(inputs):
    sh, flags = _prep_shared(inputs)
    x = np.asarray(inputs["x"])
    in_maps = []
    for c in range(NCORES):
        m = dict(sh)
        m["oh"] = _onehot(x[c * BPC:(c + 1) * BPC])
        in_maps.append(m)
    return in_maps, flags


def kernel(**inputs):
    import os
    from concourse.bass_utils import run_bass_kernel_spmd

    in_maps, flags = make_in_maps(inputs)
    nc = _get_nc(flags)
    kw = {}
    if os.environ.get("BASS_TRACE"):
        d = os.environ.get("BASS_TRACE_DIR", "/tmp/bass_trace")
        os.makedirs(d, exist_ok=True)
        kw["tmpdir"] = d
    res = run_bass_kernel_spmd(nc, in_maps, list(range(NCORES)), **kw)
    kernel._last = res
    outs = []
    for c in range(NCORES):
        lt = np.asarray(res.results[c]["logT"], np.float32)   # [V, TOK]
        outs.append(np.ascontiguousarray(lt.T).reshape(BPC, T, V))
    return np.concatenate(outs, axis=0)


kernel._last = None



# revision 5
# speedup vs baseline: 1.4230x; 1.4230x over previous
"""Trainium2 Bass kernel for a 3-layer dense transformer (BigramModel).

Contract: kernel(**inputs) takes the FULL unsharded numpy inputs (as produced
by setup_inputs) and returns the full [B, T, V] float32 logits. Internally the
batch dim B=128 is sharded 16-per-core across 8 NeuronCores (pure data
parallelism, weights replicated), one Bass/Tile NEFF run via
run_bass_kernel_spmd.

Layout strategy on device (per core, 16 seqs x 256 tok = 4096 tokens):
  - residual h: token-major fp32 SBUF tiles [128, 384] x 32 (persistent)
  - LayerNorm: DVE bn_stats/bn_aggr per tile; rstd = exp(-0.5*ln(var+eps))
    (keeps ACT in the natural_log_exp table set shared with softmax exp);
    gamma/beta are folded into the adjacent weight matrices on the host.
  - matmuls in bf16 (fp32 PSUM accumulation). Feature-major operands
    (xn^T, o^T) produced by bf16 DMA transposes (XBAR).
  - attention: per (seq, head) scores kept feature-major [s, t] so softmax
    sums run through the matmul path: V is augmented with a ones column so
    the o-matmul also produces the softmax denominators; probs are masked
    multiplicatively after exp (no -inf handling needed).
  - biases that land on free dims (bproj, b2, beta@Wv) are added inside PSUM
    via K=1 ones-row matmuls, emitted only when the host sees nonzero values.
"""

import numpy as np
import ml_dtypes

BF16 = ml_dtypes.bfloat16

P = 128
T = 256
E = 384
V = 65
H = 6
HS = 64
FF = 1536
L = 3
NCORES = 8
BPC = 16              # sequences per core
TOK = BPC * T         # 4096 tokens per core
NT = TOK // P         # 32 token tiles
NB = TOK // 512       # 8 blocks of 512 tokens (2 seqs)
ECH = E // P          # 3
FCH = FF // P         # 12

_NC_CACHE = {}
TMODE = "pe"   # "dma" = XBAR dma transpose; "pe" = TensorE transpose + copy
STAGE = 99      # debug: truncate per-block body (1=LN,2=QKV,3=probs,4=o,5=proj,6=MLP)
SUB = 2         # debug stage-3 detail (legacy)
MLPVAR = "full" # debug: "reuse" skip 2nd LN, "norelu" plain evac, "full"


def _build_nc(flags):
    """Build + compile the Bass program. flags = (bv_nz, bp_nz, b2_nz) per layer."""
    import concourse.bacc as bacc
    import concourse.mybir as mybir
    import concourse.tile as tile

    dt = mybir.dt
    f32 = dt.float32
    bf = dt.bfloat16
    Alu = mybir.AluOpType
    Act = mybir.ActivationFunctionType

    from concourse.masks import make_identity

    nc = bacc.Bacc("TRN2", target_bir_lowering=False, debug=False, num_devices=1)

    # ---- DRAM tensors (shapes match SBUF layouts; host pre-arranges) ----
    D = {}
    D["oh"] = nc.dram_tensor("oh", [V, TOK], bf, kind="ExternalInput")
    D["te"] = nc.dram_tensor("te", [V, E], bf, kind="ExternalInput")
    D["pos"] = nc.dram_tensor("pos", [P, 2, E], f32, kind="ExternalInput")
    D["mask"] = nc.dram_tensor("mask", [P, 2 * P], bf, kind="ExternalInput")
    for l in range(L):
        for w in ("wq", "wk", "wv", "wproj"):
            D[f"{w}{l}"] = nc.dram_tensor(f"{w}{l}", [P, ECH, E], bf, kind="ExternalInput")
        D[f"bq{l}"] = nc.dram_tensor(f"bq{l}", [P, ECH], f32, kind="ExternalInput")
        D[f"bk{l}"] = nc.dram_tensor(f"bk{l}", [P, ECH], f32, kind="ExternalInput")
        D[f"w1{l}"] = nc.dram_tensor(f"w1{l}", [P, ECH, FF], bf, kind="ExternalInput")
        D[f"b1c{l}"] = nc.dram_tensor(f"b1c{l}", [P, FCH], f32, kind="ExternalInput")
        D[f"w2{l}"] = nc.dram_tensor(f"w2{l}", [P, FCH, E], bf, kind="ExternalInput")
        D[f"bvrow{l}"] = nc.dram_tensor(f"bvrow{l}", [1, E], bf, kind="ExternalInput")
        D[f"bpc{l}"] = nc.dram_tensor(f"bpc{l}", [P, ECH], f32, kind="ExternalInput")
        D[f"b2c{l}"] = nc.dram_tensor(f"b2c{l}", [P, ECH], f32, kind="ExternalInput")
    if MLPVAR == "w2dump":
        D["dbga"] = nc.dram_tensor("dbga", [P, FCH, 512], bf, kind="ExternalOutput")
        D["dbgo"] = nc.dram_tensor("dbgo", [P, 4, E], f32, kind="ExternalOutput")
    D["wout"] = nc.dram_tensor("wout", [P, ECH, V], bf, kind="ExternalInput")
    D["boutc"] = nc.dram_tensor("boutc", [V, 1], f32, kind="ExternalInput")
    D["logT"] = nc.dram_tensor("logT", [V, TOK], f32, kind="ExternalOutput")

    bv_nz, bp_nz, b2_nz = flags

    with tile.TileContext(nc) as tc:
        import contextlib

        with contextlib.ExitStack() as ctx:
            const = ctx.enter_context(tc.tile_pool(name="const", bufs=1))
            wpool = ctx.enter_context(tc.tile_pool(name="wpool", bufs=1))
            act = ctx.enter_context(tc.tile_pool(name="act", bufs=3))
            act2 = ctx.enter_context(tc.tile_pool(name="act2", bufs=2))
            act1 = ctx.enter_context(tc.tile_pool(name="act1", bufs=1))
            ps_lin = ctx.enter_context(tc.tile_pool(name="ps_lin", bufs=3, space="PSUM"))
            ps_sc = ctx.enter_context(tc.tile_pool(name="ps_sc", bufs=3, space="PSUM"))
            ps_tp = ctx.enter_context(tc.tile_pool(name="ps_tp", bufs=2, space="PSUM"))

            def load_const(name, shape, dtp):
                t = const.tile(shape, dtp, tag=name)
                nc.sync.dma_start(out=t[:], in_=D[name].ap())
                return t

            # pad the K=65 embedding contraction to K=128 (sub-128 partition
            # matmuls are flaky on HW); pad rows are zeroed so they add 0.
            oh_sb = const.tile([P, TOK], bf, tag="oh")
            nc.vector.memset(oh_sb[:], 0.0)
            nc.sync.dma_start(out=oh_sb[0:V, :], in_=D["oh"].ap())
            te_sb = const.tile([P, E], bf, tag="te")
            nc.vector.memset(te_sb[:], 0.0)
            nc.sync.dma_start(out=te_sb[0:V, :], in_=D["te"].ap())
            pos_sb = load_const("pos", [P, 2, E], f32)
            mask_sb = load_const("mask", [P, 2 * P], bf)
            boutc_sb = load_const("boutc", [V, 1], f32)
            ones_sb = const.tile([1, P], bf, tag="ones")
            nc.vector.memset(ones_sb[:], 1.0)
            eps_sb = const.tile([P, 1], f32, tag="eps")
            nc.vector.memset(eps_sb[:], 1e-5)
            zero_sb = const.tile([P, 1], f32, tag="zero")
            nc.vector.memset(zero_sb[:], 0.0)
            if TMODE == "pe":
                ident_sb = const.tile([P, P], bf, tag="ident")
                make_identity(nc, ident_sb[:])
            _tp_ctr = [0]

            def tpose(dst, src):
                """dst[P,128] (sbuf bf16) = transpose(src[P,128] sbuf bf16)."""
                if TMODE == "dma":
                    nc.sync.dma_start_transpose(dst, src)
                    return
                tp = ps_tp.tile([P, P], bf, tag="tp", name="tp")
                nc.tensor.transpose(tp[:], src, ident_sb[:])
                k = _tp_ctr[0] = _tp_ctr[0] + 1
                if k % 2 == 0:
                    nc.vector.tensor_copy(out=dst, in_=tp[:])
                else:
                    nc.scalar.copy(out=dst, in_=tp[:])

            # persistent residual tiles
            h = []
            for i in range(NT):
                h.append(const.tile([P, E], f32, tag=f"h{i}", name=f"h{i}"))

            # ---- embedding: h = onehot.T @ tok_emb + pos ----
            for i in range(NT):
                ps = ps_lin.tile([P, E], f32, tag="mm")
                nc.tensor.matmul(
                    ps[:], oh_sb[:, i * P:(i + 1) * P], te_sb[:],
                    start=True, stop=True,
                )
                nc.vector.tensor_add(out=h[i][:], in0=ps[:], in1=pos_sb[:, i % 2, :])

            def ln_block(i0, tag):
                """LN of h[i0..i0+3] -> xn bf16 [P,4,E] and xnT bf16 [P,ECH,512]."""
                xn = act2.tile([P, 4, E], bf, tag="xn")
                mv4 = act.tile([P, 4, 2], f32, tag="mv")
                rstd4 = act.tile([P, 4], f32, tag="rstd")
                for j in range(4):
                    st6 = act.tile([P, 6], f32, tag="bnst")
                    nc.vector.bn_stats(out=st6[:], in_=h[i0 + j][:])
                    nc.vector.bn_aggr(out=mv4[:, j, :], in_=st6[:])
                # rstd = exp(-0.5 * ln(var + eps))
                nc.scalar.activation(
                    out=rstd4[:], in_=mv4[:, :, 1], func=Act.Ln, bias=eps_sb[:],
                )
                nc.scalar.activation(
                    out=rstd4[:], in_=rstd4[:], func=Act.Exp, scale=-0.5,
                )
                for j in range(4):
                    nc.vector.tensor_scalar(
                        out=xn[:, j, :], in0=h[i0 + j][:],
                        scalar1=mv4[:, j, 0:1], scalar2=rstd4[:, j:j + 1],
                        op0=Alu.subtract, op1=Alu.mult,
                    )
                xnT = act.tile([P, ECH, 512], bf, tag="xnT")
                for j in range(4):
                    for c in range(ECH):
                        tpose(
                            xnT[:, c, j * P:(j + 1) * P],
                            xn[:, j, c * P:(c + 1) * P],
                        )
                return xnT

            def linear_fmaj(xnT, w_sb, bias_sb, fch, tag, relu=False):
                """feature-major out [P, fch, 512] bf16 = (W^T xn^T); bias per-partition."""
                o = (act1 if fch == FCH else act2).tile([P, fch, 512], bf, tag=tag, name=tag)
                for f in range(fch):
                    ps = ps_lin.tile([P, 512], f32, tag="mm")
                    for c in range(ECH):
                        nc.tensor.matmul(
                            ps[:], w_sb[:, c, f * P:(f + 1) * P], xnT[:, c, :],
                            start=(c == 0), stop=(c == ECH - 1),
                        )
                    if relu:
                        nc.vector.tensor_scalar(
                            out=o[:, f, :], in0=ps[:],
                            scalar1=bias_sb[:, f:f + 1], scalar2=zero_sb[:],
                            op0=Alu.add, op1=Alu.max,
                        )
                    elif bias_sb is not None:
                        nc.vector.tensor_scalar_add(
                            out=o[:, f, :], in0=ps[:], scalar1=bias_sb[:, f:f + 1],
                        )
                    else:
                        nc.vector.tensor_copy(out=o[:, f, :], in_=ps[:])
                return o

            def linear_fmaj_resid(xT, w_sb, nch, bias_col, i0, tag):
                """h[i0+j] += (W^T x)_j via the feature-major matmul pattern
                (weights as lhsT), then DMA-transpose back to token-major."""
                yT = act2.tile([P, ECH, 512], bf, tag="yT", name="yT")
                for f in range(ECH):
                    ps = ps_lin.tile([P, 512], f32, tag="mm")
                    for c in range(nch):
                        nc.tensor.matmul(
                            ps[:], w_sb[:, c, f * P:(f + 1) * P], xT[:, c, :],
                            start=(c == 0), stop=(c == nch - 1),
                        )
                    nc.vector.tensor_scalar_add(
                        out=yT[:, f, :], in0=ps[:], scalar1=bias_col[:, f:f + 1])
                ytm = act2.tile([P, 4, E], bf, tag="ytm", name="ytm")
                for j in range(4):
                    for c in range(ECH):
                        tpose(
                            ytm[:, j, c * P:(c + 1) * P],
                            yT[:, c, j * P:(j + 1) * P],
                        )
                for j in range(4):
                    nc.vector.tensor_add(
                        out=h[i0 + j][:], in0=h[i0 + j][:], in1=ytm[:, j, :])

            def load_w(name, shape, dtp):
                t = wpool.tile(shape, dtp, tag=name[:-1])  # tag without layer idx
                nc.sync.dma_start(out=t[:], in_=D[name].ap())
                return t

            # ---- transformer layers ----
            for l in range(L):
                wq = load_w(f"wq{l}", [P, ECH, E], bf)
                wk = load_w(f"wk{l}", [P, ECH, E], bf)
                wv = load_w(f"wv{l}", [P, ECH, E], bf)
                wproj = load_w(f"wproj{l}", [P, ECH, E], bf)
                bq = load_w(f"bq{l}", [P, ECH], f32)
                bk = load_w(f"bk{l}", [P, ECH], f32)
                w1 = load_w(f"w1{l}", [P, ECH, FF], bf)
                b1c = load_w(f"b1c{l}", [P, FCH], f32)
                w2 = load_w(f"w2{l}", [P, FCH, E], bf)
                bvrow = load_w(f"bvrow{l}", [1, E], bf) if bv_nz[l] else None
                bpc = load_w(f"bpc{l}", [P, ECH], f32)
                b2c = load_w(f"b2c{l}", [P, ECH], f32)

                for b in range(NB):
                    i0 = 4 * b
                    # --- attention sublayer ---
                    if STAGE < 1:
                        continue
                    xnT = ln_block(i0, "a")
                    if STAGE < 2:
                        continue
                    QT = linear_fmaj(xnT, wq, bq, ECH, "QT")
                    KT = linear_fmaj(xnT, wk, bk, ECH, "KT")
                    # V token-major, ones-augmented: [P, 4, H, 65]
                    Vt = act2.tile([P, 4, H, 65], bf, tag="Vt")
                    for j in range(4):
                        ps = ps_lin.tile([P, E], f32, tag="mm")
                        for c in range(ECH):
                            nc.tensor.matmul(
                                ps[:], xnT[:, c, j * P:(j + 1) * P], wv[:, c, :],
                                start=(c == 0),
                                stop=(c == ECH - 1 and bvrow is None),
                            )
                        if bvrow is not None:
                            nc.tensor.matmul(
                                ps[:], ones_sb[:], bvrow[:], start=False, stop=True,
                            )
                        nc.vector.tensor_copy(
                            out=Vt[:, j, :, 0:64],
                            in_=ps.rearrange("p (h d) -> p h d", h=H),
                        )
                        nc.vector.memset(Vt[:, j, :, 64:65], 1.0)

                    if STAGE < 3:
                        continue
                    oT = act2.tile([P, ECH, 512], bf, tag="oT")
                    for s in range(2):      # the 2 sequences in this block
                        tb = s * 256        # col offset within the 512 block
                        probs = act2.tile([P, 2, H, 256], bf, tag="probs")
                        for st in range(2):  # s_tile (128 keys each)
                            tlo = 128 if st == 1 else 0
                            for hh in range(H):
                                c, off = divmod(hh * HS, P)
                                # each matmul gets its own offset-0 psum tile:
                                # outputs at nonzero tile offsets miscompute
                                # on HW (walrus bank mapping).
                                sc = ps_sc.tile([P, 256], f32, tag="sc", name="sc")
                                nc.tensor.matmul(
                                    sc[:, 0:256 - tlo],
                                    KT[off:off + HS, c, tb + st * P: tb + (st + 1) * P],
                                    QT[off:off + HS, c, tb + tlo: tb + 256],
                                    start=True, stop=True,
                                )
                                nc.scalar.activation(
                                    out=probs[:, st, hh, tlo:256],
                                    in_=sc[:, 0:256 - tlo],
                                    func=Act.Exp, scale=float(HS) ** -0.5,
                                )
                            if st == 0:
                                nc.vector.tensor_tensor(
                                    out=probs[:, 0], in0=probs[:, 0],
                                    in1=mask_sb[:, None, :].to_broadcast((P, H, 256)),
                                    op=Alu.mult,
                                )
                            else:
                                nc.vector.tensor_tensor(
                                    out=probs[:, 1, :, P:256],
                                    in0=probs[:, 1, :, P:256],
                                    in1=mask_sb[:, None, 0:P].to_broadcast((P, H, P)),
                                    op=Alu.mult,
                                )
                        if STAGE < 4:
                            continue
                        onorm = act2.tile([P, 2, E], bf, tag="onorm")
                        for tt in range(2):  # query tiles of this seq
                            # one single-shot matmul per (head, s-chunk), each
                            # into its own offset-0 psum tile; combine in SBUF.
                            osum = act2.tile([P, H, 65], f32, tag="osum")
                            for hh in range(H):
                                oa = ps_lin.tile([P, 65], f32, tag="mm", name="oa")
                                nc.tensor.matmul(
                                    oa[:],
                                    probs[:, 0, hh, tt * P:(tt + 1) * P],
                                    Vt[:, 2 * s, hh, :],
                                    start=True, stop=True,
                                )
                                nc.scalar.copy(out=osum[:, hh, :], in_=oa[:])
                                if tt == 1:
                                    oab = ps_lin.tile([P, 65], f32, tag="mm", name="oab")
                                    nc.tensor.matmul(
                                        oab[:],
                                        probs[:, 1, hh, P:2 * P],
                                        Vt[:, 2 * s + 1, hh, :],
                                        start=True, stop=True,
                                    )
                                    nc.vector.tensor_add(
                                        out=osum[:, hh, :], in0=osum[:, hh, :],
                                        in1=oab[:])
                            rec = act.tile([P, H], f32, tag="rec")
                            nc.vector.reciprocal(out=rec[:], in_=osum[:, :, 64])
                            nc.vector.tensor_tensor(
                                out=onorm[:, tt].rearrange("p (h d) -> p h d", h=H),
                                in0=osum[:, :, 0:64],
                                in1=rec[:, :, None].to_broadcast((P, H, HS)),
                                op=Alu.mult,
                            )
                        for tt in range(2):
                            for c in range(ECH):
                                tpose(
                                    oT[:, c, (2 * s + tt) * P:(2 * s + tt + 1) * P],
                                    onorm[:, tt, c * P:(c + 1) * P],
                                )
                    if STAGE < 5:
                        continue
                    linear_fmaj_resid(oT, wproj, ECH, bpc, i0, "p")

                    # --- MLP sublayer ---
                    if STAGE < 6:
                        continue
                    xnT2 = xnT if MLPVAR == "reuse" else ln_block(i0, "m")
                    aT = linear_fmaj(xnT2, w1, b1c, FCH, "aT",
                                     relu=(MLPVAR != "norelu"))
                    if MLPVAR == "w2dump":
                        nc.sync.dma_start(out=D["dbga"].ap(), in_=aT[:])
                        for j in range(4):
                            ps = ps_lin.tile([P, E], f32, tag="mm", name="psd")
                            for c in range(FCH):
                                lw = act.tile([P, P], bf, tag="lw", name="lw")
                                nc.vector.tensor_copy(
                                    out=lw[:], in_=aT[:, c, j * P:(j + 1) * P])
                                nc.tensor.matmul(
                                    ps[:], lw[:],
                                    w2[:, c, :],
                                    start=(c == 0), stop=(c == FCH - 1),
                                )
                            dtmp = act.tile([P, E], f32, tag="dtmp")
                            nc.vector.tensor_copy(out=dtmp[:], in_=ps[:])
                            nc.sync.dma_start(out=D["dbgo"].ap()[:, j, :], in_=dtmp[:])
                    elif MLPVAR == "dmastage":
                        aT2 = act1.tile([P, FCH, 512], bf, tag="aT2", name="aT2")
                        nc.sync.dma_start(out=aT2[:], in_=aT[:])
                        linear_fmaj_resid(aT2, w2, FCH, b2c, i0, "m")
                    elif MLPVAR != "w1only":
                        linear_fmaj_resid(aT, w2, FCH, b2c, i0, "m")

            # ---- final LN + unembed (feature-major logits) ----
            wout = wpool.tile([P, ECH, V], bf, tag="wout")
            nc.sync.dma_start(out=wout[:], in_=D["wout"].ap())
            for b in range(NB):
                xnfT = ln_block(4 * b, "f")
                ps = ps_lin.tile([V, 512], f32, tag="mm")
                for c in range(ECH):
                    nc.tensor.matmul(
                        ps[:], wout[:, c, :], xnfT[:, c, :],
                        start=(c == 0), stop=(c == ECH - 1),
                    )
                lt = act2.tile([V, 512], f32, tag="lt")
                nc.vector.tensor_scalar_add(out=lt[:], in0=ps[:], scalar1=boutc_sb[:])
                nc.sync.dma_start(
                    out=D["logT"].ap()[:, b * 512:(b + 1) * 512], in_=lt[:],
                )

    nc.compile()
    return nc


def _prep_shared(inp):
    """Host-side weight prep: layout rearrangement + LN gamma/beta folding."""
    sh = {}

    def f32(x):
        return np.asarray(x, np.float32)

    sh["te"] = np.asarray(f32(inp["tok_emb"]), BF16)                      # [V,E]
    sh["pos"] = np.ascontiguousarray(
        f32(inp["pos_emb"]).reshape(2, P, E).transpose(1, 0, 2))          # [P,2,E]
    m = np.concatenate(
        [np.triu(np.ones((P, P), np.float32)), np.ones((P, P), np.float32)], axis=1)
    sh["mask"] = np.asarray(m, BF16)                                      # [P,256]

    def tile3(w, fdim):  # [E, fdim] -> [P, ECH, fdim]
        return np.ascontiguousarray(w.reshape(ECH, P, fdim).transpose(1, 0, 2))

    def col(b, nch):  # [nch*P] -> [P, nch]
        return np.ascontiguousarray(b.reshape(nch, P).T)

    bv_nz, bp_nz, b2_nz = [], [], []
    for l in range(L):
        g1, b1_ = f32(inp["ln1_g"][l]), f32(inp["ln1_b"][l])
        g2, b2_ = f32(inp["ln2_g"][l]), f32(inp["ln2_b"][l])
        wq = f32(inp["Wq"][l]).transpose(1, 0, 2).reshape(E, E)   # head-major cols
        wk = f32(inp["Wk"][l]).transpose(1, 0, 2).reshape(E, E)
        wv = f32(inp["Wv"][l]).transpose(1, 0, 2).reshape(E, E)
        sh[f"wq{l}"] = np.asarray(tile3(g1[:, None] * wq, E), BF16)
        sh[f"wk{l}"] = np.asarray(tile3(g1[:, None] * wk, E), BF16)
        sh[f"wv{l}"] = np.asarray(tile3(g1[:, None] * wv, E), BF16)
        sh[f"bq{l}"] = col(wq.T @ b1_, ECH)
        sh[f"bk{l}"] = col(wk.T @ b1_, ECH)
        bv = wv.T @ b1_
        sh[f"bvrow{l}"] = np.asarray(bv[None, :], BF16)
        bv_nz.append(bool(np.any(bv != 0)))
        wp = f32(inp["Wproj"][l])
        sh[f"wproj{l}"] = np.asarray(tile3(wp, E), BF16)
        bp = f32(inp["bproj"][l])
        sh[f"bpc{l}"] = col(bp, ECH)
        bp_nz.append(bool(np.any(bp != 0)))
        w1 = f32(inp["W1"][l])
        sh[f"w1{l}"] = np.asarray(tile3(g2[:, None] * w1, FF), BF16)
        sh[f"b1c{l}"] = col(f32(inp["b1"][l]) + w1.T @ b2_, FCH)
        w2 = f32(inp["W2"][l])
        sh[f"w2{l}"] = np.asarray(
            w2.reshape(FCH, P, E).transpose(1, 0, 2), BF16)
        b2r = f32(inp["b2"][l])
        sh[f"b2c{l}"] = col(b2r, ECH)
        b2_nz.append(bool(np.any(b2r != 0)))

    gf, bf_ = f32(inp["lnf_g"]), f32(inp["lnf_b"])
    wo = f32(inp["Wout"])
    sh["wout"] = np.asarray(tile3(gf[:, None] * wo, V), BF16)
    sh["boutc"] = (f32(inp["bout"]) + wo.T @ bf_).reshape(V, 1)
    flags = (tuple(bv_nz), tuple(bp_nz), tuple(b2_nz))
    return sh, flags


def _onehot(xc):
    """xc: [BPC, T] ints -> [V, TOK] bf16 one-hot (feature-major)."""
    xf = np.asarray(xc, np.int64).reshape(-1)
    oh = np.zeros((V, TOK), np.float32)
    oh[xf, np.arange(TOK)] = 1.0
    return np.asarray(oh, BF16)


def _get_nc(flags):
    if flags not in _NC_CACHE:
        _NC_CACHE[flags] = _build_nc(flags)
    return _NC_CACHE[flags]


def make_in_maps(inputs):
    sh, flags = _prep_shared(inputs)
    x = np.asarray(inputs["x"])
    in_maps = []
    for c in range(NCORES):
        m = dict(sh)
        m["oh"] = _onehot(x[c * BPC:(c + 1) * BPC])
        in_maps.append(m)
    return in_maps, flags


def kernel(**inputs):
    import os
    from concourse.bass_utils import run_bass_kernel_spmd

    in_maps, flags = make_in_maps(inputs)
    nc = _get_nc(flags)
    kw = {}
    if os.environ.get("BASS_TRACE"):
        d = os.environ.get("BASS_TRACE_DIR", "/tmp/bass_trace")
        os.makedirs(d, exist_ok=True)
        kw["tmpdir"] = d
    res = run_bass_kernel_spmd(nc, in_maps, list(range(NCORES)), **kw)
    kernel._last = res
    outs = []
    for c in range(NCORES):
        lt = np.asarray(res.results[c]["logT"], np.float32)   # [V, TOK]
        outs.append(np.ascontiguousarray(lt.T).reshape(BPC, T, V))
    return np.concatenate(outs, axis=0)


kernel._last = None

